# revision 25
# baseline (speedup 1.0000x reference)
"""GraphSAGE supervised forward on 8 Trainium2 NeuronCores.

Full inputs in, full output out. Data-parallel over the B=1024 seed nodes:
128 seeds per core; neighbor rows shard as contiguous row ranges. Tiny
weights replicated.

Design — quantize + transpose on host, PE group-sums, algebraic fold:
  - hop-2 neighbors (the 82MB/core f32 stream) are sent as fp8e4m3 in
    feat-major, PHASE-MAJOR-per-tile layout; hop-1 neighbors fp8;
    seeds fp16. End-to-end max rel err ~8.5e-3 (gate 2e-2): the two
    mean-over-25 stages attenuate per-element quantization noise.
  - group-sum of 25 phases runs on the PE as accumulating identity
    matmuls: stationary [I;I] fp8 + DoubleRow packs 2 phases per
    column-slot (12 DR + 1 plain matmul per tile, all moving operands
    contiguous blocks), f32 PSUM accumulation.
  - algebraic fold #1: the hop-1 mean commutes with the aggregator
    matmul, so per-column hidden states are never materialized; only
    per-seed sums survive: redS = DVE reduce of the GS PSUM, negS =
    chunked DVE group-sum of negT.
  - algebraic fold #2: hop-1 + MLP layer 1 are linear in seedT/negS/
    redS, so six host-precomputed fp16 weight products (wtop@W1h,
    wtop@wbot@W1h, wbot@wbot@W1h per side, 1/25 folded) feed one
    six-matmul PSUM group per output part. Everything mid-stream is
    fp8/fp16 — f32 matmuls measurably halve the PE clock for
    neighboring fp8 work.
  - MLP (fp16) + softmax (f32) run in 4 parts as seed ranges complete,
    split into single-stage pieces drained between group-sum bursts so
    the in-order PE queue never sits behind a dependent chain; sides
    interleave per tile; ragged tiles (small first/last) cut pipeline
    fill and tail latency; a few throwaway warmup matmuls hold the PE
    clock up through the DMA fill phase.
Measured: 83.1us HW (baseline 288.7us): DMA-bound at ~345GB/s/core with
~8.6us startup + ~4us final part chain + ~4us drain.
"""

import sys

for _p in ("/opt/trn_rl_repo", "/root/.axon_site/_ro/trn_rl_repo"):
    if _p not in sys.path:
        sys.path.append(_p)

import numpy as np
import ml_dtypes
from contextlib import ExitStack

import concourse.bass as bass
import concourse.tile as tile
from concourse import bacc, mybir
from concourse.bass_utils import run_bass_kernel_spmd

B, S, D = 1024, 25, 128
NCORES = 8
BL = B // NCORES          # 128 seeds per core
G1 = BL * S               # 3200 hop-1 rows (= hop-2 groups) per core
G2 = G1 * S               # 80000 hop-2 rows per core

# ragged stream tiles (groups per tile, per side); sum = G1
SIZES = [100, 400, 400, 400, 400, 400, 400, 400, 200, 100]
OFFS = np.cumsum([0] + SIZES).tolist()
NTT = len(SIZES)
assert OFFS[-1] == G1 and all(sz % S == 0 for sz in SIZES)

F32 = mybir.dt.float32
F16 = mybir.dt.float16
BF16 = mybir.dt.bfloat16
F8 = mybir.dt.float8e4
AX = mybir.AxisListType
AF = mybir.ActivationFunctionType
DR = mybir.MatmulPerfMode.DoubleRow

NPF8 = ml_dtypes.float8_e4m3
NPBF = ml_dtypes.bfloat16
NPF16 = np.float16


def _build_program():
    nc = bacc.Bacc("TRN2", target_bir_lowering=False, debug=False)

    ins = {}
    for side in ("s", "d"):
        ins[f"seed_{side}"] = nc.dram_tensor(f"seed_{side}", [D, BL], F16, kind="ExternalInput")
        ins[f"neg_{side}"] = nc.dram_tensor(f"neg_{side}", [D, G1], F8, kind="ExternalInput")
        ins[f"nn_{side}"] = nc.dram_tensor(f"nn_{side}", [D, G2], F8, kind="ExternalInput")
    for name, shape, dt in (
        ("ident2", [D, 2 * D], F8),
        ("ws_seed", [D, D], F16), ("ws_neg", [D, D], F16), ("ws_red", [D, D], F16),
        ("wd_seed", [D, D], F16), ("wd_neg", [D, D], F16), ("wd_red", [D, D], F16),
        ("w2m", [D, 64], F16), ("w3m", [64, 8], F16), ("w4m", [8, 2], F16),
    ):
        ins[name] = nc.dram_tensor(name, shape, dt, kind="ExternalInput")
    out_dram = nc.dram_tensor("out", [BL, 2], F32, kind="ExternalOutput")

    with tile.TileContext(nc) as tc, ExitStack() as ctx:
        const = ctx.enter_context(tc.tile_pool(name="const", bufs=1))
        persist = ctx.enter_context(tc.tile_pool(name="persist", bufs=1))
        stream = ctx.enter_context(tc.tile_pool(name="stream", bufs=8))
        work = ctx.enter_context(tc.tile_pool(name="work", bufs=3))
        psA = ctx.enter_context(tc.tile_pool(name="psA", bufs=4, space="PSUM"))
        psM = ctx.enter_context(tc.tile_pool(name="psM", bufs=3, space="PSUM"))
        psW = ctx.enter_context(tc.tile_pool(name="psW", bufs=1, space="PSUM"))

        def load_const(name, shape, dt):
            t = const.tile(shape, dt, tag=name, name=name)
            nc.gpsimd.dma_start(t[:], ins[name].ap())
            return t

        # order matters: ident2 feeds the first stream tile's matmuls;
        # the big negT loads go last so they don't starve early stream tiles
        ident2 = load_const("ident2", [D, 2 * D], F8)
        negT, seedT = {}, {}
        for side in ("s", "d"):
            seedT[side] = const.tile([D, BL], F16, tag=f"seedT_{side}", name=f"seedT_{side}")
            nc.gpsimd.dma_start(seedT[side][:], ins[f"seed_{side}"].ap())
        wf = {}
        for side in ("s", "d"):
            for what in ("seed", "neg", "red"):
                wf[side, what] = load_const(f"w{side}_{what}", [D, D], F16)
        w2m = load_const("w2m", [D, 64], F16)
        w3m = load_const("w3m", [64, 8], F16)
        w4m = load_const("w4m", [8, 2], F16)
        for side in ("s", "d"):
            negT[side] = const.tile([D, G1], F8, tag=f"negT_{side}", name=f"negT_{side}")
            nc.gpsimd.dma_start(negT[side][:], ins[f"neg_{side}"].ap())

        idv2 = ident2.rearrange("p (j m) -> p j m", j=2)  # [128, 2, 128]
        id1 = ident2[:, 0:D]                              # [128, 128]

        negS, redS = {}, {}
        for side in ("s", "d"):
            negS[side] = persist.tile([D, BL], F16, tag=f"negS_{side}", name=f"negS_{side}")
            redS[side] = persist.tile([D, BL], F16, tag=f"redS_{side}", name=f"redS_{side}")

        PARTS = [(0, 52), (52, 84), (84, 116), (116, BL)]

        mst = {}

        def mlp_l1(pi):
            # fused hop-1 + MLP layer 1: x@W1 is linear in seedT/negS/redS,
            # so six host-precomputed weight products feed one psum group
            lo, hi = PARTS[pi]
            w = hi - lo
            ps1 = psM.tile([D, w], F32, tag="ps_m")
            srcs = [("s", "seed", seedT["s"]), ("s", "neg", negS["s"]), ("s", "red", redS["s"]),
                    ("d", "seed", seedT["d"]), ("d", "neg", negS["d"]), ("d", "red", redS["d"])]
            for i, (side, what, ten) in enumerate(srcs):
                nc.tensor.matmul(ps1[:], wf[side, what][:], ten[:, lo:hi],
                                 start=(i == 0), stop=(i == len(srcs) - 1))
            h1 = work.tile([D, w], F16, tag="h1")
            nc.scalar.activation(h1[:], ps1[:], AF.Relu)
            mst[pi, 1] = h1

        def mlp_l2(pi):
            lo, hi = PARTS[pi]
            w = hi - lo
            ps2 = psM.tile([64, w], F32, tag="ps_m")
            nc.tensor.matmul(ps2[:], w2m[:], mst[pi, 1][:])
            h2 = work.tile([64, w], F16, tag="h2")
            nc.scalar.activation(h2[:], ps2[:], AF.Relu)
            mst[pi, 2] = h2

        def mlp_l3(pi):
            lo, hi = PARTS[pi]
            w = hi - lo
            ps3 = psM.tile([8, w], F32, tag="ps_m")
            nc.tensor.matmul(ps3[:], w3m[:], mst[pi, 2][:])
            h3 = work.tile([8, w], F16, tag="h3")
            nc.scalar.activation(h3[:], ps3[:], AF.Relu)
            mst[pi, 3] = h3

        def mlp_sm(pi):
            lo, hi = PARTS[pi]
            w = hi - lo
            ps4 = psM.tile([w, 2], F32, tag="ps_m")
            nc.tensor.matmul(ps4[:], mst[pi, 3][:], w4m[:])
            lg = work.tile([w, 2], F32, tag="lg")
            nc.scalar.activation(lg[:], ps4[:], AF.Copy)
            nm = work.tile([w, 1], F32, tag="nm")
            nc.vector.reduce_max(nm[:], lg[:], axis=AX.X, negate=True)
            ex = work.tile([w, 2], F32, tag="ex")
            se = work.tile([w, 1], F32, tag="se")
            nc.scalar.activation(ex[:], lg[:], AF.Exp, bias=nm[:], accum_out=se[:])
            rc = work.tile([w, 1], F32, tag="rc")
            nc.vector.reciprocal(rc[:], se[:])
            o = work.tile([w, 2], F32, tag="o")
            nc.vector.tensor_scalar_mul(o[:], ex[:], rc[:])
            # SWDGE: a sync-queue store would head-of-line block stream loads
            nc.gpsimd.dma_start(out_dram.ap()[lo:hi], o[:])

        def warmup(n):
            # throwaway fp8 matmuls to hold the PE clock up through DMA
            # stalls in the fill phase; results land in a scratch psum
            for _ in range(n):
                pw = psW.tile([D, 2 * D], F32, tag="ps_warm")
                nc.tensor.matmul(pw[:], id1, ident2[:], start=True, stop=True)

        seeds_done = {"s": 0, "d": 0}
        next_part = [0]
        pieces = []

        def maybe_parts():
            # enqueue part piece-groups once both sides' seed sums reach a
            # boundary; one group (internally independent) drains per stream
            # tile so the in-order PE queue never sits behind a dependent
            # chain
            while next_part[0] < len(PARTS) and min(seeds_done.values()) >= PARTS[next_part[0]][1]:
                pi = next_part[0]
                pieces.extend([
                    [lambda p=pi: mlp_l1(p)],
                    [lambda p=pi: mlp_l2(p)],
                    [lambda p=pi: mlp_l3(p)],
                    [lambda p=pi: mlp_sm(p)],
                ])
                next_part[0] += 1

        def drain_pieces(k):
            for _ in range(k):
                if pieces:
                    for f in pieces.pop(0):
                        f()

        def stream_tile(side, t):
            g0, sz = OFFS[t], SIZES[t]
            xt = stream.tile([D, sz * S], F8, tag="xt", name="xt")
            nc.sync.dma_start(xt[:], ins[f"nn_{side}"].ap()[:, g0 * S:(g0 + sz) * S])
            # phase-major tile: xr[:, k, :] = phase k's sz group-columns
            xr = xt.rearrange("p (k g) -> p k g", k=S)
            ps = psA.tile([D, sz], F32, tag="ps_red")
            for i in range(S // 2):
                nc.tensor.matmul(
                    ps[:], idv2, xr[:, 2 * i:2 * i + 2, :],
                    start=(i == 0), stop=False, perf_mode=DR,
                )
            nc.tensor.matmul(ps[:], id1, xr[:, S - 1, :], start=False, stop=True)
            # per-seed sums straight from PSUM (25 group-cols per seed);
            # fp16 out: one rounding of an f32 sum, feeds the 5x-attenuated
            # mean half of hop-1
            with nc.allow_low_precision(reason="fp16 out of f32 psum sums"):
                nc.vector.reduce_sum(
                    redS[side][:, g0 // S:(g0 + sz) // S],
                    ps.rearrange("p (b s) -> p b s", s=S),
                    axis=AX.X,
                )
            seeds_done[side] = (g0 + sz) // S

        # warmup counts after each early tile's group-sum, tuned to the
        # measured DMA fill schedule (each unit ~256 cols of dummy matmul)
        WARM = {("d", 0): 5}
        for t in range(NTT):
            for side in ("s", "d"):
                stream_tile(side, t)
                if 1 <= t <= 4:
                    # negS in small chunks so the DVE queue never delays the
                    # redS reduces that recycle psA buffers
                    c0 = 2 * (t - 1)
                    with nc.allow_low_precision(reason="fp16 out of fp8 sums"):
                        nc.vector.reduce_sum(
                            negS[side][:, c0 * 16:(c0 + 2) * 16],
                            negT[side][:, c0 * 400:(c0 + 2) * 400]
                            .rearrange("p (b s) -> p b s", s=S),
                            axis=AX.X,
                        )
                warmup(WARM.get((side, t), 0))
                maybe_parts()
                drain_pieces(1)
        while pieces:
            drain_pieces(1)

    nc.compile()
    return nc


_NC_CACHE = None


def _get_program():
    global _NC_CACHE
    if _NC_CACHE is None:
        _NC_CACHE = _build_program()
    return _NC_CACHE


def kernel(src, src_neg, src_neg_neg, dst, dst_neg, dst_neg_neg, w2, W1, W2, W3, W4,
           _trace=False, **trace_kwargs):
    nc = _get_program()

    w2 = np.asarray(w2, np.float32)
    W1 = np.asarray(W1, np.float32)
    wtop = np.ascontiguousarray(w2[:D])
    wbot = np.ascontiguousarray(w2[D:]) / np.float32(S)
    eye = np.eye(D, dtype=np.float32)
    wtb = wtop @ wbot
    wbb = wbot @ wbot
    rep = {
        "ident2": np.concatenate([eye, eye], axis=1).astype(NPF8),
        "ws_seed": (wtop @ W1[:D]).astype(NPF16),
        "ws_neg": (wtb @ W1[:D]).astype(NPF16),
        "ws_red": (wbb @ W1[:D]).astype(NPF16),
        "wd_seed": (wtop @ W1[D:]).astype(NPF16),
        "wd_neg": (wtb @ W1[D:]).astype(NPF16),
        "wd_red": (wbb @ W1[D:]).astype(NPF16),
        "w2m": np.asarray(W2, NPF16),
        "w3m": np.asarray(W3, NPF16),
        "w4m": np.asarray(W4, NPF16),
    }

    def shardT(x, dt, rows):
        # [NCORES*rows, D] -> transposed per core -> [NCORES, D, rows]
        return np.ascontiguousarray(
            np.asarray(x).astype(dt).reshape(NCORES, rows, D).transpose(0, 2, 1)
        )

    def shard_nn(x):
        # [NCORES*G2, D] -> fp8, feat-major + phase-major per ragged tile:
        # out[c, f, OFFS[t]*S + k*SIZES[t] + g] = x[c*G2 + (OFFS[t]+g)*S + k, f]
        x8 = np.asarray(x).astype(NPF8).reshape(NCORES, G1, S, D)
        out = np.empty((NCORES, D, G2), NPF8)
        for t, sz in enumerate(SIZES):
            g0 = OFFS[t]
            blk = x8[:, g0:g0 + sz]                    # [C, sz, S, D]
            out[:, :, g0 * S:(g0 + sz) * S] = (
                blk.transpose(0, 3, 2, 1).reshape(NCORES, D, sz * S)
            )
        return out

    big = {
        "nn_s": shard_nn(src_neg_neg),
        "nn_d": shard_nn(dst_neg_neg),
        "neg_s": shardT(src_neg, NPF8, G1),
        "neg_d": shardT(dst_neg, NPF8, G1),
        "seed_s": shardT(src, NPF16, BL),
        "seed_d": shardT(dst, NPF16, BL),
    }
    in_maps = []
    for c in range(NCORES):
        m = dict(rep)
        for k, v in big.items():
            m[k] = v[c]
        in_maps.append(m)

    res = run_bass_kernel_spmd(
        nc, in_maps, list(range(NCORES)), trace=_trace, **trace_kwargs
    )
    out = np.concatenate([res.results[c]["out"] for c in range(NCORES)], axis=0)
    if _trace:
        return out, res
    return out
